# revision 41
# baseline (speedup 1.0000x reference)
"""Bezier-to-image Gaussian splat kernel for Trainium2 (8 NeuronCores).

Reference computation (per sample b of 256):
    T = warped cubic Bernstein basis (30, 4)
    points = einsum('nk,blkc->blnc', T, x.reshape(B,160,4,2))   # (B,160,30,2)
    gx[b,l,i,n] = exp(-(i/60 - X[b,l,n])^2 / 2e-4)
    out[b,i,j]  = min(sum_{l,n} gx[b,l,i,n]*gy[b,l,j,n], 1)     # (B,60,60)

Strategy: pure data parallel, 32 samples per core.

Input path: x is DMA'd CONTIGUOUSLY as (128, 320) (one descriptor per
partition; the previous strided 8B-element gather cost ~16us of software
descriptor generation per 8-sample group and paced the whole kernel).
Three PE transposes move the (curve-sub, ctrl-point) axis into
partitions, then 20 constant basis matmuls compute r = 256*60*coord for
ALL 4800 points x 32 samples in one go; a few strided copies compact the
PSUM result into an int16 r_all keyed [point-row, (sample, chunk, side)].

Per sample: the 4800 points live in 40 chunks of 120 (partition dim
p = lsub*30 + n, rows 120..127 dead).  The banded distance
dd[p, c*120 + w*2 + s] = 256*w - r is ONE int16 tensor_tensor whose
operands are all packed 2-byte (DVE 2x mode).  The Gaussian is a single
Derivative_Erf pass with scale SDERF/256 and a per-partition bias that
pushes dead rows to +30 (-> exactly 0).  The 60x60 image accumulates on
the Tensor engine as sum_c GxT_c^T @ GyT_c into PSUM.
"""

import math

import numpy as np
import orjson

import bass_rust
import concourse.bass as bass
import concourse.mybir as mybir
import concourse.tile as tile
from concourse.bass_utils import run_bass_kernel_spmd

B, L, N, W = 256, 160, 30, 60
NCORES = 8
BC = B // NCORES          # samples per core
ALPHA = 2e-4
KEXP = 1.0 / (W * W * ALPHA)          # exponent scale in cell units: 1/0.72
SDERF = math.sqrt(KEXP)               # Derivative_Erf input scale
DERF_FIX = math.pi / 4.0              # undo (2/sqrt(pi))^2 from Derivative_Erf
CHUNKS = 40                           # 4 curves x 30 samples per chunk
PTS = 128                             # chunk partition dim: p = lsub*30 + n,
                                      # rows 120..127 are dead
CS = 2 * CHUNKS                       # (chunk, side) slots per point row
RQ = 256.0                            # fixed-point scale for r and iota

LAST_RESULTS = None  # test harness reads profiling info from here


def _basis_T() -> np.ndarray:
    t = np.arange(N, dtype=np.float32) / np.float32(N)
    t = 2 * t**3 - 3 * t**2 + 2 * t
    t_3_0 = t**3
    t_2_1 = t**2 - t_3_0
    t_1_2 = t_3_0 - 2 * t**2 + t
    t_0_3 = (1 - t) ** 3
    return np.stack([t_3_0, 3 * t_2_1, 3 * t_1_2, t_0_3], axis=1).astype(np.float32)


def _legalize_waits(nc, max_waits: int = 1):
    """Walrus rejects engine instructions carrying more than ~1 sync wait
    ("Too many sync wait commands").  Hoist excess waits onto same-engine
    Drain instructions inserted immediately before the offender (the same
    carrier the Tile epilogue barrier uses, observed with up to 7 waits)."""
    js = orjson.loads(mybir.module_to_json_bytes(nc.m))
    ctr = 0
    for f in js["functions"]:
        for bb in f["blocks"]:
            out = []
            changed = False
            for inst in bb["instructions"]:
                si = inst.get("sync_info")
                waits = si.get("on_wait") if si else None
                if waits and len(waits) > max_waits:
                    keep = waits[:max_waits]
                    for w in waits[max_waits:]:
                        ctr += 1
                        out.append({
                            "debug": inst.get("debug", 0),
                            "engine": inst["engine"],
                            "ins": [], "outs": [],
                            "name": f"waitfix-{ctr}",
                            "opcode": "Drain",
                            "sync_info": {"on_update": [], "on_wait": [w]},
                        })
                    si["on_wait"] = keep
                    changed = True
                out.append(inst)
            if changed:
                bb["instructions"] = out
    if ctr:
        nc.m = bass_rust.module_from_json_bytes(orjson.dumps(js))
    return ctr


def _basis_C() -> np.ndarray:
    """(8, 128, 128) f32: C[lq*2+co, p', m] with p' = lsub_r*8 + kc and
    m = lsub''*30 + n.  Nonzero (= 256*60*T[n, kc//2]) iff kc%2 == co and
    lsub_r == lq*4 + lsub''.  Contracting C against the transposed x piece
    yields r for curve l = q*40 + 16F + 4*lq + lsub'', point n, coord co."""
    T = _basis_T()  # (30, 4)
    C = np.zeros((8, 128, 128), dtype=np.float32)
    for lq in range(4):
        for co in range(2):
            for lsub2 in range(4):
                lsub_r = lq * 4 + lsub2
                for k in range(4):
                    kc = 2 * k + co
                    C[lq * 2 + co, lsub_r * 8 + kc,
                      lsub2 * N : (lsub2 + 1) * N] = RQ * W * T[:, k]
    return C


def build_program(legalize: bool = True):
    f32 = mybir.dt.float32
    f16 = mybir.dt.float16
    i16 = mybir.dt.int16

    nc = bass.Bass("TRN2", target_bir_lowering=False, debug=False)

    x_t = nc.dram_tensor("x", [BC, L, 8], f32, kind="ExternalInput")
    y_t = nc.dram_tensor("y", [BC, W, W], f32, kind="ExternalOutput")

    C_np = _basis_C()
    C_d = nc.inline_tensor(
        C_np.transpose(1, 0, 2).reshape(128, 8 * 128).copy(), name="basisC"
    )
    ident_d = nc.inline_tensor(np.eye(128, dtype=np.float32), name="ident")

    # iota for one chunk of the (chunk, w, side)-interleaved band layout:
    # value 256*w at offset w*2 + s; broadcast over the chunk dim with a
    # stride-0 outer AP dim (only the innermost must be packed for 2x mode)
    iota_np = np.broadcast_to(
        (RQ * np.arange(W, dtype=np.float32)).astype(np.int16)[None, :, None],
        (PTS, W, 2),
    ).reshape(PTS, 2 * W).copy()
    iota_d = nc.inline_tensor(iota_np, name="iotaw")

    # per-partition activation bias: dead rows 120..127 get +30 so the
    # Derivative_Erf input is far out of range -> exactly 0
    bias_np = np.zeros((PTS, 1), dtype=np.float32)
    bias_np[120:] = 30.0
    bias_d = nc.inline_tensor(bias_np, name="abias")

    # --- wrapped-band mode constants (used on a few samples to offload the
    # Activation engine, the steady-state bottleneck).  wb[m] evaluates the
    # Gaussian on the 8-cell residue ring: uw = (256*m + 1024 - r) mod 2048,
    # wb = derf(SDERF*(uw-1024)/256), then the dense band is reconstructed as
    # wb[w%8] * [|256w - r| <= 1021].  The window test uses a shifted iota
    # (256w + 1021) and an UNSIGNED compare, so out-of-window negatives wrap
    # to huge values and a single is_le does the two-sided test.
    iotw_np = np.broadcast_to(
        (RQ * np.arange(W, dtype=np.float32) + 1021).astype(np.int16)[None, :, None],
        (PTS, W, 2),
    ).reshape(PTS, 2 * W).copy()
    iotw_d = nc.inline_tensor(iotw_np, name="iotaw2")
    iota8_np = np.broadcast_to(
        (RQ * np.arange(8, dtype=np.float32) + 19456.0).astype(np.int16)[None, :, None],
        (PTS, 8, 2),
    ).reshape(PTS, 16).copy()
    iota8_d = nc.inline_tensor(iota8_np, name="iota8")
    bias3_np = np.full((PTS, 1), -4.0 * SDERF, dtype=np.float32)
    bias3_np[120:] += 30.0
    bias3_d = nc.inline_tensor(bias3_np, name="abias3")

    # samples evaluated in wrapped-band mode (tuned to balance ACT vs DVE)
    WRAPPED = {4, 10, 16, 22, 28}

    with tile.TileContext(nc) as tc, tc.tile_pool(name="const", bufs=1) as cpool, \
            tc.tile_pool(name="outp", bufs=1) as out_pool, \
            tc.tile_pool(name="band", bufs=2) as band_pool, \
            tc.tile_pool(name="wrap", bufs=2) as wpool, \
            tc.tile_pool(name="tpsum", bufs=1, space="PSUM") as t_pool, \
            tc.tile_pool(name="rpsum", bufs=1, space="PSUM") as rps_pool, \
            tc.tile_pool(name="imgpsum", bufs=2, space="PSUM") as img_pool:

        # ---- input: one contiguous DMA, partition = (b, l-quarter).
        # Issued FIRST: the serialized constant loads behind it would
        # otherwise delay the whole r pipeline by ~7us.
        x_sb = cpool.tile([128, 320], f32, tag="xsb")
        nc.gpsimd.dma_start(
            x_sb[:],
            x_t.ap().rearrange("b l k -> (b l k)").rearrange("(p f) -> p f", p=128),
        )
        Cs = cpool.tile([128, 8 * 128], f32, tag="basisC")
        nc.gpsimd.dma_start(Cs[:], C_d.ap())
        ident = cpool.tile([128, 128], f32, tag="ident")
        nc.gpsimd.dma_start(ident[:], ident_d.ap())
        iot = cpool.tile([PTS, 2 * W], i16, tag="iota")
        nc.gpsimd.dma_start(iot[:], iota_d.ap())
        abias = cpool.tile([PTS, 1], f32, tag="abias")
        nc.gpsimd.dma_start(abias[:], bias_d.ap())
        iotw = cpool.tile([PTS, 2 * W], i16, tag="iotaw2")
        nc.gpsimd.dma_start(iotw[:], iotw_d.ap())
        iota8 = cpool.tile([PTS, 16], i16, tag="iota8")
        nc.gpsimd.dma_start(iota8[:], iota8_d.ap())
        abias3 = cpool.tile([PTS, 1], f32, tag="abias3")
        nc.gpsimd.dma_start(abias3[:], bias3_d.ap())

        # preload the Derivative_Erf table during the prologue so the first
        # real activation doesn't pay the 1.3us table swap
        warm = cpool.tile([PTS, 1], f16, tag="warm")
        nc.scalar.activation(
            warm[:], abias[:],
            mybir.ActivationFunctionType.Derivative_Erf,
            bias=0.0, scale=1.0,
        )

        # ---- r for every point of every sample, in one shot ----
        # 3 PE transposes put (lsub, kc) into partitions ...
        import os as _os
        _SKIP_T = _os.environ.get("KERNEL_SKIP_TRANSPOSE", "0") == "1"
        t_sb = cpool.tile([128, 3 * 128], f32, tag="tsb")
        if _SKIP_T:
            nc.vector.memset(t_sb[:], 0.25)
        for F in range(3):
            if _SKIP_T:
                break
            pw = 128 if F < 2 else 64
            tp = t_pool.tile([pw, 128], f32, tag="tp")
            nc.tensor.transpose(tp[:], x_sb[:, 128 * F : 128 * F + pw], ident[:])
            nc.vector.tensor_copy(t_sb[:pw, 128 * F : 128 * (F + 1)], tp[:])
        # ... then constant matmuls produce r_psum[m, idx*128 + (b,q)].
        # Split into two 16-sample phases so phase 1 overlaps early compute.
        r_ps = rps_pool.tile([128, 20 * 128], f32, tag="rps")
        r_all = cpool.tile([PTS, BC * CS], i16, tag="rall")
        ra_v = r_all[:].rearrange("p (b c s) -> p b c s", b=BC, s=2)

        # phase 0 covers samples 0..7 (small, on the critical path to the
        # first activation); phase 1 covers 8..31 under early compute
        PH = [(0, 32, 0, 8), (32, 96, 8, 24)]

        def emit_r_mms(h):
            bq0, nbq, b0, nb = PH[h]
            for F in range(3):
                pw = 128 if F < 2 else 64
                nlq = 4 if F < 2 else 2
                for lq in range(nlq):
                    for co in range(2):
                        idx = F * 8 + lq * 2 + co
                        nc.tensor.matmul(
                            r_ps[:, 128 * idx + bq0 : 128 * idx + bq0 + nbq],
                            lhsT=Cs[:pw, 128 * (lq * 2 + co) : 128 * (lq * 2 + co) + 128],
                            rhs=t_sb[:pw, 128 * F + bq0 : 128 * F + bq0 + nbq],
                            start=True,
                            stop=True,
                        )
        def emit_r_copy(h, F):
            # compact into int16 r_all[p, b*80 + c*2 + s]:
            #   c = F*16 + q*4 + lq (F<2),  32 + q*2 + lq (F=2)
            b0, nb = PH[h][2], PH[h][3]
            if F < 2:
                nc.vector.tensor_copy(
                    ra_v[:, b0 : b0 + nb, 16 * F : 16 * (F + 1), :]
                    .rearrange("p b (q lq) s -> p b q lq s", q=4),
                    r_ps[:, 1024 * F : 1024 * (F + 1)]
                    .rearrange("p (lq co b q) -> p b q lq co", lq=4, co=2, q=4)
                    [:, b0 : b0 + nb],
                )
            else:
                nc.vector.tensor_copy(
                    ra_v[:, b0 : b0 + nb, 32:40, :]
                    .rearrange("p b (q lq) s -> p b q lq s", q=4),
                    r_ps[:, 2048:2560]
                    .rearrange("p (lq co b q) -> p b q lq co", lq=2, co=2, q=4)
                    [:, b0 : b0 + nb],
                )

        emit_r_mms(0)
        for F in range(3):
            emit_r_copy(0, F)

        # all 32 output images live here until the final DMA
        out_all = out_pool.tile([W, BC * W], f32, tag="oall")

        def emit_min_and_store(j):
            # min(scale*img, 1) -> staging; stream finished 4-sample blocks
            nc.vector.tensor_scalar(
                out_all[:, W * j : W * (j + 1)],
                imgs[j],
                DERF_FIX,
                1.0,
                mybir.AluOpType.mult,
                mybir.AluOpType.min,
            )
            if j % 4 == 3:
                g = j // 4
                nc.sync.dma_start(
                    y_t.ap()[4 * g : 4 * (g + 1)].rearrange("b i j -> i b j"),
                    out_all[:, W * 4 * g : W * 4 * (g + 1)]
                    .rearrange("i (b j) -> i b j", b=4),
                )

        # Emission units: adjacent non-wrapped samples are PAIRED — their
        # banded distances land in one double-width tile and a single
        # activation covers both, amortizing the ~0.3us per-instruction
        # Activation overhead (the bottleneck engine).
        units = []
        b = 0
        while b < BC:
            if b in WRAPPED:
                units.append((b,))
                b += 1
            elif b + 1 < BC and b + 1 not in WRAPPED:
                units.append((b, b + 1))
                b += 2
            else:
                units.append((b,))
                b += 1

        def emit_dd(dst_view, b):
            nc.vector.tensor_tensor(
                dst_view.rearrange("p (c w s) -> p c w s", w=W, s=2),
                iot[:].rearrange("p (o w s) -> p o w s", o=1, s=2)
                .broadcast_to([PTS, CHUNKS, W, 2]),
                r_all[:, CS * b : CS * (b + 1)]
                .rearrange("p (c o s) -> p c o s", o=1, s=2)
                .broadcast_to([PTS, CHUNKS, W, 2]),
                mybir.AluOpType.subtract,
            )

        imgs = {}
        pending = []
        SZ = W * CS
        for ui, unit in enumerate(units):
            if ui == 1:
                emit_r_mms(1)
            elif ui in (2, 3, 4):
                emit_r_copy(1, ui - 2)
            b = unit[0]
            if b not in WRAPPED:
                # ---- banded distance, int16: dd[p, c*120+w*2+s] = 256*w - r.
                # All operands are packed 2-byte with stride-1 innermost dims
                # so the DVE runs in its 2x perf mode.
                dd = band_pool.tile([PTS, 2 * SZ], i16, tag="dd")
                gg = band_pool.tile([PTS, 2 * SZ], f16, tag="gg")
                for k, bb in enumerate(unit):
                    emit_dd(dd[:, k * SZ : (k + 1) * SZ], bb)
                # ---- Gaussian: Derivative_Erf(dd*SDERF/256 + bias) in fp16;
                # bias is +30 on dead partitions -> 0 there.
                n = len(unit)
                nc.scalar.activation(
                    gg[:, : n * SZ], dd[:, : n * SZ],
                    mybir.ActivationFunctionType.Derivative_Erf,
                    bias=abias[:], scale=SDERF / RQ,
                )
            else:
                # ---- wrapped-band mode: Gaussian on the 8-cell residue ring
                # (640 ACT elements instead of 4800), dense reconstruction on
                # the DVE.  Offloads the bottleneck Activation engine.
                r_b = r_all[:, CS * b : CS * (b + 1)]
                r_bc = (
                    r_b.rearrange("p (c o s) -> p c o s", o=1, s=2)
                    .broadcast_to([PTS, CHUNKS, W, 2])
                )
                tw = wpool.tile([PTS, 16 * CHUNKS], i16, tag="tw")
                nc.vector.tensor_tensor(
                    tw[:].rearrange("p (c m s) -> p c m s", m=8, s=2),
                    iota8[:].rearrange("p (o m s) -> p o m s", o=1, m=8)
                    .broadcast_to([PTS, CHUNKS, 8, 2]),
                    r_b.rearrange("p (c o s) -> p c o s", o=1, s=2)
                    .broadcast_to([PTS, CHUNKS, 8, 2]),
                    mybir.AluOpType.subtract,
                )
                uw = wpool.tile([PTS, 16 * CHUNKS], i16, tag="uw")
                nc.vector.tensor_scalar(
                    uw[:], tw[:], 2047, None, mybir.AluOpType.bitwise_and
                )
                wb = wpool.tile([PTS, 16 * CHUNKS], f16, tag="wb")
                nc.scalar.activation(
                    wb[:], uw[:],
                    mybir.ActivationFunctionType.Derivative_Erf,
                    bias=abias3[:], scale=SDERF / RQ,
                )
                ddf = band_pool.tile([PTS, 2 * SZ], i16, tag="dd")
                ddw = ddf[:, :SZ]
                nc.vector.tensor_tensor(
                    ddw.rearrange("p (c w s) -> p c w s", w=W, s=2),
                    iotw[:].rearrange("p (o w s) -> p o w s", o=1, s=2)
                    .broadcast_to([PTS, CHUNKS, W, 2]),
                    r_bc,
                    mybir.AluOpType.subtract,
                )
                mk = wpool.tile([PTS, W * CS], f16, tag="mk")
                nc.vector.tensor_scalar(
                    mk[:], ddw.bitcast(mybir.dt.uint16), 2042, None,
                    mybir.AluOpType.is_le,
                )
                gg = band_pool.tile([PTS, 2 * SZ], f16, tag="gg")
                ggw = gg[:, :SZ]
                wb_bc = wb[:].rearrange(
                    "p (c o ms) -> p c o ms", o=1, ms=16
                ).broadcast_to([PTS, CHUNKS, 8, 16])
                # w splits as 7 full blocks of 8 residues + a 4-residue tail
                nc.vector.tensor_tensor(
                    ggw.rearrange("p (c x) -> p c x", c=CHUNKS)[:, :, 0:112]
                    .rearrange("p c (q ms) -> p c q ms", ms=16),
                    mk[:].rearrange("p (c x) -> p c x", c=CHUNKS)[:, :, 0:112]
                    .rearrange("p c (q ms) -> p c q ms", ms=16),
                    wb_bc[:, :, 0:7],
                    mybir.AluOpType.mult,
                )
                nc.vector.tensor_tensor(
                    ggw.rearrange("p (c x) -> p c x", c=CHUNKS)[:, :, 112:120],
                    mk[:].rearrange("p (c x) -> p c x", c=CHUNKS)[:, :, 112:120],
                    wb[:].rearrange("p (c ms) -> p c ms", ms=16)[:, :, 0:8],
                    mybir.AluOpType.mult,
                )

            # ---- image accumulation: sum_c GxT_c^T @ GyT_c ----
            for k, bb in enumerate(unit):
                gv = gg[:, k * SZ : (k + 1) * SZ].rearrange(
                    "p (c w s) -> p c w s", w=W, s=2
                )
                img = img_pool.tile([W, W], f32, tag="img")
                imgs[bb] = img[:]
                for c in range(CHUNKS):
                    nc.tensor.matmul(
                        img[:],
                        lhsT=gv[:, c, :, 0],
                        rhs=gv[:, c, :, 1],
                        start=(c == 0),
                        stop=(c == CHUNKS - 1),
                    )

            # software pipelining: earlier samples' mins are emitted AFTER
            # this unit's subtracts in the Vector stream, so the next
            # subtract never queues behind a min waiting on matmuls
            for bb in pending:
                emit_min_and_store(bb)
            pending = list(unit)

        for bb in pending:
            emit_min_and_store(bb)

    if legalize:
        _legalize_waits(nc)
    return nc


_PROGRAM = None


def kernel(x: np.ndarray, _trace: bool = False) -> np.ndarray:
    global _PROGRAM, LAST_RESULTS
    assert x.shape == (B, L, 8) and x.dtype == np.float32, (x.shape, x.dtype)
    if _PROGRAM is None:
        _PROGRAM = build_program()
    nc = _PROGRAM
    shards = np.split(np.ascontiguousarray(x), NCORES, axis=0)
    in_maps = [{"x": s} for s in shards]
    res = run_bass_kernel_spmd(nc, in_maps, list(range(NCORES)), trace=_trace)
    LAST_RESULTS = res
    return np.concatenate([res.results[i]["y"] for i in range(NCORES)], axis=0)


# revision 47
# speedup vs baseline: 1.1837x; 1.1837x over previous
"""Bezier-to-image Gaussian splat kernel for Trainium2 (8 NeuronCores).

Reference computation (per sample b of 256):
    T = warped cubic Bernstein basis (30, 4)
    points = einsum('nk,blkc->blnc', T, x.reshape(B,160,4,2))   # (B,160,30,2)
    gx[b,l,i,n] = exp(-(i/60 - X[b,l,n])^2 / 2e-4)
    out[b,i,j]  = min(sum_{l,n} gx[b,l,i,n]*gy[b,l,j,n], 1)     # (B,60,60)

Strategy: pure data parallel, 32 samples per core.

Input path: x is DMA'd CONTIGUOUSLY as (128, 320) (one descriptor per
partition; the previous strided 8B-element gather cost ~16us of software
descriptor generation per 8-sample group and paced the whole kernel).
Three PE transposes move the (curve-sub, ctrl-point) axis into
partitions, then 20 constant basis matmuls compute r = 256*60*coord for
ALL 4800 points x 32 samples in one go; a few strided copies compact the
PSUM result into an int16 r_all keyed [point-row, (sample, chunk, side)].

Per sample: the 4800 points live in 40 chunks of 120 (partition dim
p = lsub*30 + n, rows 120..127 dead).  The banded distance
dd[p, c*120 + w*2 + s] = 256*w - r is ONE int16 tensor_tensor whose
operands are all packed 2-byte (DVE 2x mode).  The Gaussian is a single
Derivative_Erf pass with scale SDERF/256 and a per-partition bias that
pushes dead rows to +30 (-> exactly 0).  The 60x60 image accumulates on
the Tensor engine as sum_c GxT_c^T @ GyT_c into PSUM.
"""

import math

import numpy as np
import orjson

import bass_rust
import concourse.bass as bass
import concourse.mybir as mybir
import concourse.tile as tile
from concourse.bass_utils import run_bass_kernel_spmd

B, L, N, W = 256, 160, 30, 60
NCORES = 8
BC = B // NCORES          # samples per core
ALPHA = 2e-4
KEXP = 1.0 / (W * W * ALPHA)          # exponent scale in cell units: 1/0.72
SDERF = math.sqrt(KEXP)               # Derivative_Erf input scale
DERF_FIX = math.pi / 4.0              # undo (2/sqrt(pi))^2 from Derivative_Erf
CHUNKS = 40                           # 4 curves x 30 samples per chunk
PTS = 128                             # chunk partition dim: p = lsub*30 + n,
                                      # rows 120..127 are dead
CS = 2 * CHUNKS                       # (chunk, side) slots per point row
RQ = 256.0                            # fixed-point scale for r and iota

LAST_RESULTS = None  # test harness reads profiling info from here


def _basis_T() -> np.ndarray:
    t = np.arange(N, dtype=np.float32) / np.float32(N)
    t = 2 * t**3 - 3 * t**2 + 2 * t
    t_3_0 = t**3
    t_2_1 = t**2 - t_3_0
    t_1_2 = t_3_0 - 2 * t**2 + t
    t_0_3 = (1 - t) ** 3
    return np.stack([t_3_0, 3 * t_2_1, 3 * t_1_2, t_0_3], axis=1).astype(np.float32)


def _legalize_waits(nc, max_waits: int = 1):
    """Walrus rejects engine instructions carrying more than ~1 sync wait
    ("Too many sync wait commands").  Hoist excess waits onto same-engine
    Drain instructions inserted immediately before the offender (the same
    carrier the Tile epilogue barrier uses, observed with up to 7 waits)."""
    js = orjson.loads(mybir.module_to_json_bytes(nc.m))
    ctr = 0
    for f in js["functions"]:
        for bb in f["blocks"]:
            out = []
            changed = False
            for inst in bb["instructions"]:
                si = inst.get("sync_info")
                waits = si.get("on_wait") if si else None
                if waits and len(waits) > max_waits:
                    keep = waits[:max_waits]
                    for w in waits[max_waits:]:
                        ctr += 1
                        out.append({
                            "debug": inst.get("debug", 0),
                            "engine": inst["engine"],
                            "ins": [], "outs": [],
                            "name": f"waitfix-{ctr}",
                            "opcode": "Drain",
                            "sync_info": {"on_update": [], "on_wait": [w]},
                        })
                    si["on_wait"] = keep
                    changed = True
                out.append(inst)
            if changed:
                bb["instructions"] = out
    if ctr:
        nc.m = bass_rust.module_from_json_bytes(orjson.dumps(js))
    return ctr


def _basis_C() -> np.ndarray:
    """(8, 128, 128) f32: C[lq*2+co, p', m] with p' = lsub_r*8 + kc and
    m = lsub''*30 + n.  Nonzero (= 256*60*T[n, kc//2]) iff kc%2 == co and
    lsub_r == lq*4 + lsub''.  Contracting C against the transposed x piece
    yields r for curve l = q*40 + 16F + 4*lq + lsub'', point n, coord co."""
    T = _basis_T()  # (30, 4)
    C = np.zeros((8, 128, 128), dtype=np.float32)
    for lq in range(4):
        for co in range(2):
            for lsub2 in range(4):
                lsub_r = lq * 4 + lsub2
                for k in range(4):
                    kc = 2 * k + co
                    C[lq * 2 + co, lsub_r * 8 + kc,
                      lsub2 * N : (lsub2 + 1) * N] = RQ * W * T[:, k]
    return C


def build_program(legalize: bool = True):
    f32 = mybir.dt.float32
    f16 = mybir.dt.float16
    i16 = mybir.dt.int16

    nc = bass.Bass("TRN2", target_bir_lowering=False, debug=False)

    x_t = nc.dram_tensor("x", [BC, L, 8], f32, kind="ExternalInput")
    y_t = nc.dram_tensor("y", [BC, W, W], f32, kind="ExternalOutput")

    C_np = _basis_C()
    C_d = nc.inline_tensor(
        C_np.transpose(1, 0, 2).reshape(128, 8 * 128).copy(), name="basisC"
    )
    ident_d = nc.inline_tensor(np.eye(128, dtype=np.float32), name="ident")

    # iota for one chunk of the (chunk, w, side)-interleaved band layout:
    # value 256*w at offset w*2 + s; broadcast over the chunk dim with a
    # stride-0 outer AP dim (only the innermost must be packed for 2x mode)
    iota_np = np.broadcast_to(
        (RQ * np.arange(W, dtype=np.float32)).astype(np.int16)[None, :, None],
        (PTS, W, 2),
    ).reshape(PTS, 2 * W).copy()
    iota_d = nc.inline_tensor(iota_np, name="iotaw")

    # per-partition activation bias: dead rows 120..127 get +30 so the
    # Derivative_Erf input is far out of range -> exactly 0
    bias_np = np.zeros((PTS, 1), dtype=np.float32)
    bias_np[120:] = 30.0
    bias_d = nc.inline_tensor(bias_np, name="abias")

    # --- wrapped-band mode constants (used on a few samples to offload the
    # Activation engine, the steady-state bottleneck).  wb[m] evaluates the
    # Gaussian on the 8-cell residue ring: uw = (256*m + 1024 - r) mod 2048,
    # wb = derf(SDERF*(uw-1024)/256), then the dense band is reconstructed as
    # wb[w%8] * [|256w - r| <= 1021].  The window test uses a shifted iota
    # (256w + 1021) and an UNSIGNED compare, so out-of-window negatives wrap
    # to huge values and a single is_le does the two-sided test.
    iotw_np = np.broadcast_to(
        (RQ * np.arange(W, dtype=np.float32) + 1021).astype(np.int16)[None, :, None],
        (PTS, W, 2),
    ).reshape(PTS, 2 * W).copy()
    iotw_d = nc.inline_tensor(iotw_np, name="iotaw2")
    iota8_np = np.broadcast_to(
        (RQ * np.arange(8, dtype=np.float32) + 19456.0).astype(np.int16)[None, :, None],
        (PTS, 8, 2),
    ).reshape(PTS, 16).copy()
    iota8_d = nc.inline_tensor(iota8_np, name="iota8")
    bias3_np = np.full((PTS, 1), -4.0 * SDERF, dtype=np.float32)
    bias3_np[120:] += 30.0
    bias3_d = nc.inline_tensor(bias3_np, name="abias3")

    # samples evaluated in wrapped-band mode (tuned to balance ACT vs DVE)
    WRAPPED = {4, 10, 16, 22, 28}

    with tile.TileContext(nc) as tc, tc.tile_pool(name="const", bufs=1) as cpool, \
            tc.tile_pool(name="outp", bufs=1) as out_pool, \
            tc.tile_pool(name="band", bufs=4) as band_pool, \
            tc.tile_pool(name="wrap", bufs=2) as wpool, \
            tc.tile_pool(name="tpsum", bufs=1, space="PSUM") as t_pool, \
            tc.tile_pool(name="rpsum", bufs=1, space="PSUM") as rps_pool, \
            tc.tile_pool(name="imgpsum", bufs=2, space="PSUM") as img_pool:

        # ---- input: one contiguous DMA, partition = (b, l-quarter).
        # Issued FIRST: the serialized constant loads behind it would
        # otherwise delay the whole r pipeline by ~7us.
        x_sb = cpool.tile([128, 320], f32, tag="xsb")
        nc.gpsimd.dma_start(
            x_sb[:],
            x_t.ap().rearrange("b l k -> (b l k)").rearrange("(p f) -> p f", p=128),
        )
        Cs = cpool.tile([128, 8 * 128], f32, tag="basisC")
        nc.gpsimd.dma_start(Cs[:], C_d.ap())
        ident = cpool.tile([128, 128], f32, tag="ident")
        nc.gpsimd.dma_start(ident[:], ident_d.ap())
        iot = cpool.tile([PTS, 2 * W], i16, tag="iota")
        nc.gpsimd.dma_start(iot[:], iota_d.ap())
        abias = cpool.tile([PTS, 1], f32, tag="abias")
        nc.gpsimd.dma_start(abias[:], bias_d.ap())
        iotw = cpool.tile([PTS, 2 * W], i16, tag="iotaw2")
        nc.gpsimd.dma_start(iotw[:], iotw_d.ap())
        iota8 = cpool.tile([PTS, 16], i16, tag="iota8")
        nc.gpsimd.dma_start(iota8[:], iota8_d.ap())
        abias3 = cpool.tile([PTS, 1], f32, tag="abias3")
        nc.gpsimd.dma_start(abias3[:], bias3_d.ap())

        # preload the Derivative_Erf table during the prologue so the first
        # real activation doesn't pay the 1.3us table swap
        warm = cpool.tile([PTS, 1], f16, tag="warm")
        nc.scalar.activation(
            warm[:], abias[:],
            mybir.ActivationFunctionType.Derivative_Erf,
            bias=0.0, scale=1.0,
        )

        # ---- r for every point of every sample, in one shot ----
        # 3 PE transposes put (lsub, kc) into partitions ...
        import os as _os
        _SKIP_T = _os.environ.get("KERNEL_SKIP_TRANSPOSE", "0") == "1"
        t_sb = cpool.tile([128, 3 * 128], f32, tag="tsb")
        if _SKIP_T:
            nc.vector.memset(t_sb[:], 0.25)
        for F in range(3):
            if _SKIP_T:
                break
            pw = 128 if F < 2 else 64
            tp = t_pool.tile([pw, 128], f32, tag="tp")
            nc.tensor.transpose(tp[:], x_sb[:, 128 * F : 128 * F + pw], ident[:])
            nc.vector.tensor_copy(t_sb[:pw, 128 * F : 128 * (F + 1)], tp[:])
        # ... then constant matmuls produce r_psum[m, idx*128 + (b,q)].
        # Split into two 16-sample phases so phase 1 overlaps early compute.
        r_ps = rps_pool.tile([128, 20 * 128], f32, tag="rps")
        r_all = cpool.tile([PTS, BC * CS], i16, tag="rall")
        ra_v = r_all[:].rearrange("p (b c s) -> p b c s", b=BC, s=2)

        # phase 0 covers samples 0..7 (small, on the critical path to the
        # first activation); phase 1 covers 8..31 under early compute
        PH = [(0, 32, 0, 8), (32, 96, 8, 24)]

        def emit_r_mms(h):
            bq0, nbq, b0, nb = PH[h]
            for F in range(3):
                pw = 128 if F < 2 else 64
                nlq = 4 if F < 2 else 2
                for lq in range(nlq):
                    for co in range(2):
                        idx = F * 8 + lq * 2 + co
                        nc.tensor.matmul(
                            r_ps[:, 128 * idx + bq0 : 128 * idx + bq0 + nbq],
                            lhsT=Cs[:pw, 128 * (lq * 2 + co) : 128 * (lq * 2 + co) + 128],
                            rhs=t_sb[:pw, 128 * F + bq0 : 128 * F + bq0 + nbq],
                            start=True,
                            stop=True,
                        )
        def emit_r_copy(h, F):
            # compact into int16 r_all[p, b*80 + c*2 + s]:
            #   c = F*16 + q*4 + lq (F<2),  32 + q*2 + lq (F=2)
            b0, nb = PH[h][2], PH[h][3]
            if F < 2:
                nc.vector.tensor_copy(
                    ra_v[:, b0 : b0 + nb, 16 * F : 16 * (F + 1), :]
                    .rearrange("p b (q lq) s -> p b q lq s", q=4),
                    r_ps[:, 1024 * F : 1024 * (F + 1)]
                    .rearrange("p (lq co b q) -> p b q lq co", lq=4, co=2, q=4)
                    [:, b0 : b0 + nb],
                )
            else:
                nc.vector.tensor_copy(
                    ra_v[:, b0 : b0 + nb, 32:40, :]
                    .rearrange("p b (q lq) s -> p b q lq s", q=4),
                    r_ps[:, 2048:2560]
                    .rearrange("p (lq co b q) -> p b q lq co", lq=2, co=2, q=4)
                    [:, b0 : b0 + nb],
                )

        emit_r_mms(0)
        for F in range(3):
            emit_r_copy(0, F)

        # all 32 output images live here until the final DMA
        out_all = out_pool.tile([W, BC * W], f32, tag="oall")

        def emit_min_and_store(j):
            # min(scale*img, 1) -> staging; stream finished 4-sample blocks
            nc.vector.tensor_scalar(
                out_all[:, W * j : W * (j + 1)],
                imgs[j],
                DERF_FIX,
                1.0,
                mybir.AluOpType.mult,
                mybir.AluOpType.min,
            )
            if j % 4 == 3:
                g = j // 4
                nc.sync.dma_start(
                    y_t.ap()[4 * g : 4 * (g + 1)].rearrange("b i j -> i b j"),
                    out_all[:, W * 4 * g : W * 4 * (g + 1)]
                    .rearrange("i (b j) -> i b j", b=4),
                )

        units = [(b,) for b in range(BC)]

        def emit_dd(dst_view, b):
            nc.vector.tensor_tensor(
                dst_view.rearrange("p (c w s) -> p c w s", w=W, s=2),
                iot[:].rearrange("p (o w s) -> p o w s", o=1, s=2)
                .broadcast_to([PTS, CHUNKS, W, 2]),
                r_all[:, CS * b : CS * (b + 1)]
                .rearrange("p (c o s) -> p c o s", o=1, s=2)
                .broadcast_to([PTS, CHUNKS, W, 2]),
                mybir.AluOpType.subtract,
            )

        imgs = {}
        pending = []
        SZ = W * CS
        for ui, unit in enumerate(units):
            if ui == 2:
                emit_r_mms(1)
            elif ui in (3, 4, 5):
                emit_r_copy(1, ui - 3)
            b = unit[0]
            if b not in WRAPPED:
                # ---- banded distance, int16: dd[p, c*120+w*2+s] = 256*w - r.
                # All operands are packed 2-byte with stride-1 innermost dims
                # so the DVE runs in its 2x perf mode.
                dd = band_pool.tile([PTS, SZ], i16, tag="dd")
                gg = band_pool.tile([PTS, SZ], f16, tag="gg")
                emit_dd(dd[:], b)
                # ---- Gaussian: Derivative_Erf(dd*SDERF/256 + bias) in fp16;
                # bias is +30 on dead partitions -> 0 there.
                nc.scalar.activation(
                    gg[:], dd[:],
                    mybir.ActivationFunctionType.Derivative_Erf,
                    bias=abias[:], scale=SDERF / RQ,
                )
            else:
                # ---- wrapped-band mode: Gaussian on the 8-cell residue ring
                # (640 ACT elements instead of 4800), dense reconstruction on
                # the DVE.  Offloads the bottleneck Activation engine.
                r_b = r_all[:, CS * b : CS * (b + 1)]
                r_bc = (
                    r_b.rearrange("p (c o s) -> p c o s", o=1, s=2)
                    .broadcast_to([PTS, CHUNKS, W, 2])
                )
                tw = wpool.tile([PTS, 16 * CHUNKS], i16, tag="tw")
                nc.vector.tensor_tensor(
                    tw[:].rearrange("p (c m s) -> p c m s", m=8, s=2),
                    iota8[:].rearrange("p (o m s) -> p o m s", o=1, m=8)
                    .broadcast_to([PTS, CHUNKS, 8, 2]),
                    r_b.rearrange("p (c o s) -> p c o s", o=1, s=2)
                    .broadcast_to([PTS, CHUNKS, 8, 2]),
                    mybir.AluOpType.subtract,
                )
                uw = wpool.tile([PTS, 16 * CHUNKS], i16, tag="uw")
                nc.vector.tensor_scalar(
                    uw[:], tw[:], 2047, None, mybir.AluOpType.bitwise_and
                )
                wb = wpool.tile([PTS, 16 * CHUNKS], f16, tag="wb")
                nc.scalar.activation(
                    wb[:], uw[:],
                    mybir.ActivationFunctionType.Derivative_Erf,
                    bias=abias3[:], scale=SDERF / RQ,
                )
                ddf = band_pool.tile([PTS, SZ], i16, tag="dd")
                ddw = ddf[:]
                nc.vector.tensor_tensor(
                    ddw.rearrange("p (c w s) -> p c w s", w=W, s=2),
                    iotw[:].rearrange("p (o w s) -> p o w s", o=1, s=2)
                    .broadcast_to([PTS, CHUNKS, W, 2]),
                    r_bc,
                    mybir.AluOpType.subtract,
                )
                mk = wpool.tile([PTS, W * CS], f16, tag="mk")
                nc.vector.tensor_scalar(
                    mk[:], ddw.bitcast(mybir.dt.uint16), 2042, None,
                    mybir.AluOpType.is_le,
                )
                gg = band_pool.tile([PTS, SZ], f16, tag="gg")
                ggw = gg[:]
                wb_bc = wb[:].rearrange(
                    "p (c o ms) -> p c o ms", o=1, ms=16
                ).broadcast_to([PTS, CHUNKS, 8, 16])
                # w splits as 7 full blocks of 8 residues + a 4-residue tail
                nc.vector.tensor_tensor(
                    ggw.rearrange("p (c x) -> p c x", c=CHUNKS)[:, :, 0:112]
                    .rearrange("p c (q ms) -> p c q ms", ms=16),
                    mk[:].rearrange("p (c x) -> p c x", c=CHUNKS)[:, :, 0:112]
                    .rearrange("p c (q ms) -> p c q ms", ms=16),
                    wb_bc[:, :, 0:7],
                    mybir.AluOpType.mult,
                )
                nc.vector.tensor_tensor(
                    ggw.rearrange("p (c x) -> p c x", c=CHUNKS)[:, :, 112:120],
                    mk[:].rearrange("p (c x) -> p c x", c=CHUNKS)[:, :, 112:120],
                    wb[:].rearrange("p (c ms) -> p c ms", ms=16)[:, :, 0:8],
                    mybir.AluOpType.mult,
                )

            # ---- image accumulation: sum_c GxT_c^T @ GyT_c ----
            for k, bb in enumerate(unit):
                gv = gg[:, k * SZ : (k + 1) * SZ].rearrange(
                    "p (c w s) -> p c w s", w=W, s=2
                )
                img = img_pool.tile([W, W], f32, tag="img")
                imgs[bb] = img[:]
                for c in range(CHUNKS):
                    nc.tensor.matmul(
                        img[:],
                        lhsT=gv[:, c, :, 0],
                        rhs=gv[:, c, :, 1],
                        start=(c == 0),
                        stop=(c == CHUNKS - 1),
                    )

            # software pipelining: earlier samples' mins are emitted AFTER
            # this unit's subtracts in the Vector stream, so the next
            # subtract never queues behind a min waiting on matmuls
            for bb in pending:
                emit_min_and_store(bb)
            pending = list(unit)

        for bb in pending:
            emit_min_and_store(bb)

    if legalize:
        _legalize_waits(nc)
    return nc


_PROGRAM = None


def kernel(x: np.ndarray, _trace: bool = False) -> np.ndarray:
    global _PROGRAM, LAST_RESULTS
    assert x.shape == (B, L, 8) and x.dtype == np.float32, (x.shape, x.dtype)
    if _PROGRAM is None:
        _PROGRAM = build_program()
    nc = _PROGRAM
    shards = np.split(np.ascontiguousarray(x), NCORES, axis=0)
    in_maps = [{"x": s} for s in shards]
    res = run_bass_kernel_spmd(nc, in_maps, list(range(NCORES)), trace=_trace)
    LAST_RESULTS = res
    return np.concatenate([res.results[i]["y"] for i in range(NCORES)], axis=0)
